# revision 15
# baseline (speedup 1.0000x reference)
"""MoE head (top-2 of 8 experts, GELU MLP, residual + LayerNorm) on 8 trn2
NeuronCores.

Strategy (expert-parallel):
  - Host: router (logits -> top-2 -> softmax), exactly as the reference
    computes it (fp32). Tokens are gathered per expert into capacity-padded
    buffers (capacity adapts to the actual max expert load, so nothing is
    ever dropped).
  - Device (8 cores, SPMD, core e owns expert e): y_e = gelu(x_e @ W1_e
    + b1_e) @ W2_e * combine_weight, split into two phases:
      phase 1 (GEMM1): fp8-e4m3 operands with perf_mode=DoubleRow (2 fp8
        weights per PE cell -> 2 MACs/cycle, halves the matmul count).
        Inputs are pre-scaled on the host (x*16, W1*8) to lift the
        operands out of e4m3's subnormal range; the 1/128 descale folds
        into the gelu activation's input scale for free.  gelu output
        (hT) is written bf16 and stays fully SBUF-resident.
      phase 2 (GEMM2): bf16 matmuls accumulating over the full F=4096
        contraction in PSUM; epilogue is a single vector multiply by the
        combine weight (b2 is folded in on the host).
    fp8 on BOTH gemms would breach the 2e-2 gate (measured 2.3e-2 in
    simulation); GEMM1-fp8 + GEMM2-bf16 lands at ~1.67e-2.
  - All device inputs are laid out partition-major on the host so every
    DMA line is 4-16KB contiguous per partition (small strided lines ran
    at a fraction of peak and serialized ~9us of startup).  Activations /
    W1 / W2 split across the three DMA queues (sync / act / gpsimd).
  - A burst of tiny self-matmuls warms the PE clock (HAM un-throttle)
    while the first input DMAs are still in flight.
  - Host: scatter-add the two expert contributions per token (pure
    unshard/combine), b2 bias, residual add + LayerNorm -> [B, T, H].

Self-contained: hardcodes the nn_MoEHead problem shapes
(B=2, T=2048, H=1024, F=4096, E=8, top-2).
"""

import os
import sys
import types

import ml_dtypes
import numpy as np


def _ensure_axon_ntff_hook():
    """bass_utils' axon trace path does `from antenv.axon_hooks import ...`;
    the container's antenv stub lacks that submodule, which would make any
    BASS_TRACE=1 run crash.  Recreate it, wiring the ctypes NTFF profiler
    hook from trn_agent_boot when available."""
    if "antenv.axon_hooks" in sys.modules:
        return
    mod = types.ModuleType("antenv.axon_hooks")
    hook = None
    try:
        from trn_agent_boot.trn_boot import _ntff_profile_via_ctypes

        so = "/opt/axon/libaxon_pjrt.so"
        if os.path.exists(so):
            hook = _ntff_profile_via_ctypes(so)
    except Exception:
        hook = None
    mod._hook = hook
    mod.get_axon_ntff_profile_hook = lambda: mod._hook

    def _set(h):
        mod._hook = h

    mod.set_axon_ntff_profile_hook = _set
    sys.modules["antenv.axon_hooks"] = mod
    try:
        import antenv

        antenv.axon_hooks = mod
    except Exception:
        pass


_ensure_axon_ntff_hook()

import concourse.bass as bass  # noqa: E402
import concourse.tile as tile  # noqa: E402
from concourse import bacc, mybir  # noqa: E402
from concourse.bass_utils import run_bass_kernel_spmd  # noqa: E402

P = 128
H = 1024
F = 4096
E = 8
TOP_K = 2
LN_EPS = 1e-5
KO = H // P  # 8   k-tiles for GEMM1 (contraction over H)
FO = F // P  # 32  f-tiles (contraction for GEMM2)
HO = H // P  # 8   h-tiles of the output
TOK_B = 512  # max token block (psum free-dim limit for fp32)
N_W1C = 16  # W1 shipped in 16 chunk-major f-range chunks (2 f-tiles each)
FT_PER_C = FO // N_W1C  # 4
FCH = F // N_W1C  # 512

# GEMM1 dtype: "f8" = e4m3 + DoubleRow (2x PE rate), "bf16" = plain bf16.
G1_DT = os.environ.get("MOE_G1_DT", "f8")
# Number of GEMM2 f-tiles (of 32, even) computed in fp8-e4m3 DoubleRow;
# the rest stay bf16.  Each pair of converted f-tiles saves ~3.1us but
# adds quantization error: measured rel-err 1.67e-2 (b=0), 1.77e-2 (b=4),
# 1.81e-2 (b=6), 1.86e-2 (b=8) against the 2e-2 gate.
G2B = int(os.environ.get("MOE_G2_F8T", "8"))
WARMUP_MMS = int(os.environ.get("MOE_WARMUP", "16"))
# Host-side power-of-2 pre-scales, lifting e4m3 operands out of the
# subnormal range (descale folds into gelu scale / combine weights).
# W2 is scaled x16 uniformly (both the fp8 and bf16 f-tiles, keeping the
# PSUM accumulation uniformly scaled); the 1/16 folds into the combine
# weights.
SX = 16.0  # x
SW1 = 8.0  # W1
SW2 = 16.0  # W2

_kernel_cache: dict = {}
_wprep_cache: dict = {}


def _tok_blocks(C):
    """Split C tokens (a multiple of 16) into near-equal 16-aligned blocks
    of <=512 — wide moving operands keep the PE at full rate, and
    16-element alignment keeps the ISA happy."""
    assert C % 16 == 0
    nb = max(1, -(-C // TOK_B))
    n16 = C // 16
    sizes = [16 * (n16 // nb + (1 if i < n16 % nb else 0)) for i in range(nb)]
    blocks = []
    off = 0
    for sz in sizes:
        blocks.append((off, sz))
        off += sz
    return blocks


def _mm_dt(tag):
    return mybir.dt.float8e4 if tag == "f8" else mybir.dt.bfloat16


def _np_dt(tag):
    return ml_dtypes.float8_e4m3 if tag == "f8" else ml_dtypes.bfloat16


def _build_moe_kernel(C, g1, b2t):
    """One expert's FFN over C capacity-padded tokens.

    in : xT{i} per token block [P, KO, sz] (partition-major, pre-scaled +
         quantized for g1), w1 [N_W1C, P, KO, FCH] (chunk-major), b1v [F],
         w28 [P, b2t, H] fp8 / w2b [P, FO-b2t, H] bf16 (both pre-scaled
         x16), wgs [C] (combine weights, pre-descaled)
    out: yT [H, C] = (gelu(x @ W1 + b1) @ W2).T * wgt
    """
    f32 = mybir.dt.float32
    f8 = mybir.dt.float8e4
    bf16 = mybir.dt.bfloat16
    d1 = _mm_dt(g1)
    dr1 = g1 == "f8"
    ks1 = 2 if dr1 else 1  # k-tiles consumed per GEMM1 matmul
    DR = mybir.MatmulPerfMode.DoubleRow
    nc = bacc.Bacc(None, target_bir_lowering=False, debug=False)

    blocks = _tok_blocks(C)

    xTs = [
        nc.dram_tensor(f"xT{bi}", [P, KO, sz], d1, kind="ExternalInput")
        for bi, (off, sz) in enumerate(blocks)
    ]
    w1 = nc.dram_tensor("w1", [N_W1C, P, KO, FCH], d1, kind="ExternalInput")
    b1v = nc.dram_tensor("b1v", [P, FO], f32, kind="ExternalInput")
    if b2t:
        w28 = nc.dram_tensor("w28", [P, b2t, H], f8, kind="ExternalInput")
    w2b = nc.dram_tensor("w2b", [P, FO - b2t, H], bf16, kind="ExternalInput")
    wgs = nc.dram_tensor("wgs", [C], f32, kind="ExternalInput")
    yT = nc.dram_tensor("yT", [H, C], bf16, kind="ExternalOutput")

    yT_r = yT.rearrange("(ho p) c -> p ho c", p=P)  # [128, 8, C]

    g1_scale = 1.0 / (SX * SW1) if dr1 else 1.0

    with tile.TileContext(nc) as tc:
        with (
            tc.tile_pool(name="singles", bufs=1) as singles,
            tc.tile_pool(name="yp", bufs=4) as yp,
            tc.tile_pool(name="ps1", bufs=4, space="PSUM") as ps1,
            tc.tile_pool(name="ps2", bufs=4, space="PSUM") as ps2,
        ):
            # ---- DMA in; every transfer is multi-KB-contiguous per
            # partition.  sync queue: W1 chunks (chunk 0 gates the first
            # LDWEIGHTS).  act queue: activations block-by-block (GEMM1
            # sweeps block-outer so only block 0 gates the start), then W2
            # behind.  gpsimd queue: b1 + combine weights.
            xT_sbs = [
                singles.tile([P, KO, sz], d1, name=f"xT{bi}")
                for bi, (off, sz) in enumerate(blocks)
            ]
            nc.sync.dma_start(xT_sbs[0][:], xTs[0][:])
            # w1 chunk 0 rides the act queue so it transfers in parallel
            # with xT block 0 on sync — together they gate the first matmul
            w1_sb = singles.tile([P, N_W1C, KO, FCH], d1, name="w1")
            nc.scalar.dma_start(w1_sb[:, 0], w1[0])
            for bi in range(1, len(blocks)):
                nc.scalar.dma_start(xT_sbs[bi][:], xTs[bi][:])

            # warm-up operand: zeroed fp8/bf16 scratch, no DMA dependency
            wsrc = singles.tile([P, 2, 384], d1, name="wsrc")
            nc.gpsimd.memset(wsrc[:], 0)
            b1_sb = singles.tile([P, FO], f32)
            nc.gpsimd.dma_start(out=b1_sb[:], in_=b1v[:])
            wgt_sb = singles.tile([P, C], f32)
            wgt_ap = wgs[:]
            wgt_bc = bass.AP(
                tensor=wgt_ap.tensor,
                offset=wgt_ap.offset,
                ap=[[0, P], *wgt_ap.ap],
            )
            nc.gpsimd.dma_start(out=wgt_sb[:], in_=wgt_bc)

            for ci in range(1, N_W1C):
                nc.sync.dma_start(w1_sb[:, ci], w1[ci])

            # W2 is only read in phase 2 (>70us in): park it on the sync
            # queue behind the W1 chunks, keeping the act engine's
            # instruction stream clear for the phase-1 gelus.
            if b2t:
                w28_sb = singles.tile([P, b2t, H], f8, name="w28")
                nc.sync.dma_start(w28_sb[:], w28[:])
            w2b_sb = singles.tile([P, FO - b2t, H], bf16, name="w2b")
            nfb = FO - b2t
            for ci in range(4):
                lo, hi = ci * nfb // 4, (ci + 1) * nfb // 4
                nc.sync.dma_start(w2b_sb[:, lo:hi, :], w2b[:, lo:hi, :])

            # gelu output, fully SBUF-resident (first b2t f-tiles in fp8
            # for the DoubleRow portion of GEMM2, the rest bf16)
            if b2t:
                hT8 = singles.tile([P, b2t, C], f8, name="hT8")
            hTb = singles.tile([P, FO - b2t, C], bf16, name="hTb")

            # ---- PE warm-up: a burst of self-matmuls on the zeroed
            # scratch while the first input DMAs are in flight.  Results go
            # to a scratch psum that is never read; ~3.4us of PE activity
            # un-throttles the HAM clock gate (cold = half-rate) before the
            # first real matmul issues.
            if WARMUP_MMS:
                scratch = ps1.tile([P, TOK_B], f32, name="psum")
                for wi in range(WARMUP_MMS):
                    nc.tensor.matmul(
                        scratch[:64, :384],
                        wsrc[:, 0, :64],
                        wsrc[:, 1, :],
                        start=(wi == 0),
                        stop=(wi == WARMUP_MMS - 1),
                        skip_group_check=True,
                    )

            # ---- phase 1: hT[f, tok] = gelu(x @ W1 + b1), block-outer ----
            for bi, (off, sz) in enumerate(blocks):
                for ft in range(FO):
                    ci, fl = ft // FT_PER_C, (ft % FT_PER_C) * P
                    psum = ps1.tile([P, TOK_B], f32)
                    for k in range(0, KO, ks1):
                        if dr1:
                            lhsT = w1_sb[:, ci, k : k + 2, fl : fl + P]
                            rhs = xT_sbs[bi][:, k : k + 2, :]
                        else:
                            lhsT = w1_sb[:, ci, k, fl : fl + P]
                            rhs = xT_sbs[bi][:, k, :]
                        nc.tensor.matmul(
                            psum[:, :sz],
                            lhsT,
                            rhs,
                            start=(k == 0),
                            stop=(k + ks1 == KO),
                            perf_mode=DR if dr1 else None,
                        )
                    htgt = (
                        hT8[:, ft, off : off + sz]
                        if ft < b2t
                        else hTb[:, ft - b2t, off : off + sz]
                    )
                    nc.scalar.activation(
                        htgt,
                        psum[:, :sz],
                        mybir.ActivationFunctionType.Gelu,
                        bias=b1_sb[:, ft : ft + 1],
                        scale=g1_scale,
                    )

            # ---- phase 2: yT[h, tok] = (hT.T @ W2) * wgt, full-F
            # accumulation in PSUM ----
            for ho in range(HO):
                hl = ho * P
                for off, sz in blocks:
                    psum2 = ps2.tile([P, TOK_B], f32)
                    for j in range(0, b2t, 2):
                        nc.tensor.matmul(
                            psum2[:, :sz],
                            w28_sb[:, j : j + 2, hl : hl + P],
                            hT8[:, j : j + 2, off : off + sz],
                            start=(j == 0),
                            stop=False,
                            perf_mode=DR,
                        )
                    for fo in range(b2t, FO):
                        nc.tensor.matmul(
                            psum2[:, :sz],
                            w2b_sb[:, fo - b2t, hl : hl + P],
                            hTb[:, fo - b2t, off : off + sz],
                            start=(fo == 0),
                            stop=(fo == FO - 1),
                        )
                    ysb = yp.tile([P, TOK_B], bf16, name="ysb")
                    nc.vector.tensor_mul(
                        ysb[:, :sz], psum2[:, :sz], wgt_sb[:, off : off + sz]
                    )
                    nc.sync.dma_start(yT_r[:, ho, off : off + sz], ysb[:, :sz])

    nc.compile()
    return nc


def _get_kernel(C, g1, b2t):
    key = (C, g1, b2t)
    if key not in _kernel_cache:
        _kernel_cache[key] = _build_moe_kernel(C, g1, b2t)
    return _kernel_cache[key]


def _route(x, router_w, router_b):
    """Replicates the reference router bit-for-bit up to fp32 matmul
    rounding: logits -> top-2 (ties to lower index) -> softmax."""
    logits = x @ router_w.T + router_b  # [N, E] fp32
    order = np.argsort(-logits, axis=-1, kind="stable")
    idx = order[:, :TOP_K]  # [N, 2]
    vals = np.take_along_axis(logits, idx, axis=-1)
    vmax = vals.max(axis=-1, keepdims=True)
    ex = np.exp(vals - vmax)
    w = ex / ex.sum(axis=-1, keepdims=True)
    return idx, w.astype(np.float32)


def _q(a, tag, scale):
    """Quantize a*scale to the matmul dtype (e4m3 clipped to TRN's +-240
    max, or bf16); returns the raw quantized array (still carrying scale)."""
    a = np.asarray(a, np.float32)
    if tag == "f8":
        if scale != 1.0:
            a = a * np.float32(scale)
        return np.clip(a, -240.0, 240.0).astype(ml_dtypes.float8_e4m3)
    return a.astype(ml_dtypes.bfloat16)


def _prep_weights(W1, W2):
    """Per-expert quantized, partition-major weight arrays (memoized on
    array identity — the harness calls kernel() repeatedly with the same
    arrays)."""
    key = (id(W1), id(W2), G1_DT, G2B)
    if _wprep_cache.get("key") != key:
        w1q = []
        w28q = []
        w2bq = []
        bs = G2B * P
        for e in range(E):
            q1 = _q(W1[e], G1_DT, SW1)  # [H, F]
            # chunk-major [N_W1C, P, KO, FCH]: per partition each chunk is
            # KO*FCH contiguous bytes
            q1 = q1.reshape(KO, P, N_W1C, FCH).transpose(2, 1, 0, 3)
            w1q.append(np.ascontiguousarray(q1))
            w2s = np.asarray(W2[e], np.float32)  # [F, H]
            if G2B:
                q28 = _q(w2s[:bs], "f8", SW2)  # fp8(W2*16)
                w28q.append(
                    np.ascontiguousarray(q28.reshape(G2B, P, H).transpose(1, 0, 2))
                )
            q2b = (w2s[bs:] * np.float32(SW2)).astype(ml_dtypes.bfloat16)
            w2bq.append(
                np.ascontiguousarray(q2b.reshape(FO - G2B, P, H).transpose(1, 0, 2))
            )
        _wprep_cache["key"] = key
        _wprep_cache["val"] = (w1q, w28q, w2bq)
    return _wprep_cache["val"]


def kernel(
    hidden_states,
    router_w,
    router_b,
    W1,
    b1,
    W2,
    b2,
    ln_gamma,
    ln_beta,
):
    hidden_states = np.asarray(hidden_states, np.float32)
    router_w = np.asarray(router_w, np.float32)
    router_b = np.asarray(router_b, np.float32)
    b1 = np.asarray(b1, np.float32)
    b2 = np.asarray(b2, np.float32)
    ln_gamma = np.asarray(ln_gamma, np.float32)
    ln_beta = np.asarray(ln_beta, np.float32)

    B, T, Hdim = hidden_states.shape
    N = B * T
    x = np.ascontiguousarray(hidden_states.reshape(N, Hdim))

    idx, topw = _route(x, router_w, router_b)

    tok_ids = np.arange(N)
    toks_per_e = []
    wts_per_e = []
    for e in range(E):
        sel0 = idx[:, 0] == e
        sel1 = idx[:, 1] == e
        toks = np.concatenate([tok_ids[sel0], tok_ids[sel1]])
        ws = np.concatenate([topw[sel0, 0], topw[sel1, 1]])
        toks_per_e.append(toks)
        wts_per_e.append(ws)

    max_cnt = max(len(t) for t in toks_per_e)
    # capacity: multiple of 16 keeps DMA rows 64B-aligned; >=256 keeps the
    # PE at full rate
    C = max(((max_cnt + 15) // 16) * 16, 256)

    nc = _get_kernel(C, G1_DT, G2B)
    w1q, w28q, w2bq = _prep_weights(W1, W2)

    # quantize activations once, gather per expert in the narrow dtype
    xq = _q(x, G1_DT, SX if G1_DT == "f8" else 1.0)  # [N, H]
    wg_scale = 1.0 / SW2  # undo the uniform W2 x16

    blocks = _tok_blocks(C)
    in_maps = []
    for e in range(E):
        toks = toks_per_e[e]
        n = len(toks)
        X = np.zeros((C, Hdim), dtype=xq.dtype)
        X[:n] = xq[toks]
        wv = np.zeros((C,), dtype=np.float32)
        wv[:n] = wts_per_e[e] * np.float32(wg_scale)
        # per token block, partition-major [P, KO, sz]: per partition
        # KO*sz contiguous bytes (one fat DMA line each)
        xT = X.T  # [H, C]
        im = {
            "w1": w1q[e],
            "b1v": np.ascontiguousarray(b1[e].reshape(FO, P).T),
            "w2b": w2bq[e],
            "wgs": wv,
        }
        if G2B:
            im["w28"] = w28q[e]
        for bi, (off, sz) in enumerate(blocks):
            im[f"xT{bi}"] = np.ascontiguousarray(
                xT[:, off : off + sz].reshape(KO, P, sz).transpose(1, 0, 2)
            )
        in_maps.append(im)

    res = run_bass_kernel_spmd(nc, in_maps, core_ids=list(range(E)))

    out = np.zeros((N, Hdim), dtype=np.float64)
    for e in range(E):
        toks = toks_per_e[e]
        n = len(toks)
        yT = res.results[e]["yT"]  # [H, C]
        out[toks] += yT.T[:n].astype(np.float64)
        if b2[e].any():
            # b2 is applied on the host: each pair contributes b2[e]*wgt
            out[toks] += wts_per_e[e][:, None].astype(np.float64) * b2[e]

    # residual + LayerNorm (float64 internally; reference is fp32)
    out += x.astype(np.float64)
    mu = out.mean(axis=-1, keepdims=True)
    var = out.var(axis=-1, keepdims=True)
    out = (out - mu) / np.sqrt(var + LN_EPS)
    out = out * np.asarray(ln_gamma, np.float64) + np.asarray(ln_beta, np.float64)

    return out.astype(np.float32).reshape(B, T, Hdim)
